# revision 8
# baseline (speedup 1.0000x reference)
"""LIF neuron with soft reset — Trainium2 Bass kernel, 8-way data parallel.

Problem: x (T=32, B=16, C=128, H=32, W=32) f32. Sequential scan over T:
    mem = 0.25*mem + x[t];  s[t] = (mem >= 1);  mem -= s[t]
Returns spikes (same shape, f32 values in {0,1}).

Sharding: batch dim B=16 split 2-per-core across 8 NeuronCores; the scan is
elementwise over (B,C,H,W) so cores are fully independent. Per-core slice of
one timestep = 2*128*32*32 = 262144 contiguous floats -> [128 x 2048] f32.

Scaled-threshold formulation (v2): track Q_t = 4^t * mem_post_t. With
y_t = 4^t * x_t pre-scaled on the host (exact: power-of-two multiply),
    P_t = Q_{t-1} + y_t            (DMA accum-add on load, no engine cycles)
    s_t = (P_t >= 4^t)             (spike; threshold grows instead of mem
                                    decaying -- kills the per-step *0.25 op)
    Q_t = P_t - 4^t * s_t          (DMA accum-add of sbar = -(4^t)*s_t)
All rescalings are powers of two, which commute with IEEE-754 rounding, so
every intermediate is bit-identical to the reference's fp32 sequence.

Per-step engine assignment (per column-group, G independent groups pipeline
the serial recurrence):
  SWDGE  : Q += y_t (DRAM->SBUF accum), Q += sbar (SBUF->SBUF accum)
  VectorE: sbar = (Q is_ge 4^t) mult -4^t   (single dual-op tensor_scalar, 2x)
  ScalarE: su8 = Copy(-4^-t * sbar) -> u8   (spike output convert)
  HWDGE  : store su8 (u8 = 4x less HBM write traffic; host upcasts to f32)
"""

import os

import numpy as np

T, B, C, H, W = 32, 16, 128, 32, 32
NCORES = 8
BPC = B // NCORES          # batches per core
P = 128                    # SBUF partitions
F = (BPC * C * H * W) // P # 2048 free-dim elements per step
G = int(os.environ.get("LIF_GROUPS", "4"))   # independent column groups
FG = F // G

_cache = {}


def _build(reps: int = 1):
    import concourse.bacc as bacc
    import concourse.mybir as mybir
    from concourse.tile import TileContext

    nc = bacc.Bacc(None, target_bir_lowering=False)
    x_d = nc.dram_tensor("x", [T, P, F], mybir.dt.float32, kind="ExternalInput")
    o_d = nc.dram_tensor("o", [T, P, F], mybir.dt.uint8, kind="ExternalOutput")

    fp32 = mybir.dt.float32
    u8 = mybir.dt.uint8
    Alu = mybir.AluOpType
    Act = mybir.ActivationFunctionType

    with TileContext(nc) as tc:
        with (
            tc.tile_pool(name="mem", bufs=1) as mempool,
            tc.tile_pool(name="spk", bufs=3) as spool,
            tc.tile_pool(name="out", bufs=3) as opool,
        ):
            q = [mempool.tile([P, FG], fp32, name=f"q{g}", tag=f"q{g}") for g in range(G)]
            for _ in range(reps):  # reps>1 only for benchmarking
                for g in range(G):
                    nc.vector.memset(q[g], 0.0)
                for t in range(T):
                    thr = float(4.0 ** t)
                    for g in range(G):
                        cols = slice(g * FG, (g + 1) * FG)
                        # P_t = Q + 4^t x_t  (accum-add during the load)
                        nc.gpsimd.dma_start(
                            out=q[g], in_=x_d[t][:, cols], accum_op=Alu.add
                        )
                        # sbar = -(4^t) * (P_t >= 4^t)
                        sb = spool.tile([P, FG], fp32, name=f"s{g}", tag=f"s{g}")
                        nc.vector.tensor_scalar(
                            sb, q[g], thr, -thr, Alu.is_ge, Alu.mult
                        )
                        # spike out: u8 = Copy(-4^-t * sbar) in {0,1}
                        su = opool.tile([P, FG], u8, name=f"u{g}", tag=f"u{g}")
                        nc.scalar.activation(
                            su, sb, Act.Copy, bias=0.0, scale=-(1.0 / thr)
                        )
                        nc.sync.dma_start(out=o_d[t][:, cols], in_=su)
                        # soft reset: Q_t = P_t + sbar
                        nc.gpsimd.dma_start(out=q[g], in_=sb, accum_op=Alu.add)
    nc.finalize()
    return nc


def kernel(x: np.ndarray) -> np.ndarray:
    from concourse.bass_utils import run_bass_kernel_spmd

    assert x.shape == (T, B, C, H, W) and x.dtype == np.float32
    if "nc" not in _cache:
        _cache["nc"] = _build()
    nc = _cache["nc"]

    # host-side pre-scale: y_t = 4^t * x_t (exact power-of-two multiply)
    scale = (4.0 ** np.arange(T, dtype=np.float64)).astype(np.float32)
    y = x * scale[:, None, None, None, None]

    in_maps = []
    for k in range(NCORES):
        yk = np.ascontiguousarray(y[:, k * BPC : (k + 1) * BPC]).reshape(T, P, F)
        in_maps.append({"x": yk})

    res = run_bass_kernel_spmd(nc, in_maps, core_ids=list(range(NCORES)))
    _cache["last_result"] = res

    out = np.empty((T, B, C, H, W), dtype=np.float32)
    for k in range(NCORES):
        ok = res.results[k]["o"].reshape(T, BPC, C, H, W)
        out[:, k * BPC : (k + 1) * BPC] = ok.astype(np.float32)
    return out


# revision 9
# speedup vs baseline: 10.7578x; 10.7578x over previous
"""LIF neuron with soft reset — Trainium2 Bass kernel, 8-way data parallel.

Problem: x (T=32, B=16, C=128, H=32, W=32) f32. Sequential scan over T:
    mem = 0.25*mem + x[t];  s[t] = (mem >= 1);  mem -= s[t]
Returns spikes (same shape, f32 values in {0,1}).

Sharding: batch dim B=16 split 2-per-core across 8 NeuronCores; the scan is
elementwise over (B,C,H,W) so cores are fully independent. Per-core slice of
one timestep = 2*128*32*32 = 262144 contiguous floats -> [128 x 2048] f32.

Scaled-threshold formulation: carry PRE-reset state P_t = 4^t * mem_pre_t,
with y_t = 4^t * x_t pre-scaled on the host (exact power-of-two multiply).
    P_t   = (P_{t-1} - 4^{t-1} * [P_{t-1} >= 4^{t-1}]) + y_t
    s_t   = [P_t >= 4^t]
Power-of-two rescaling commutes with IEEE-754 rounding; the reset subtract
is exact (mem < 2^24 means mem-1 stays on mem's grid); the only rounding per
step is the +y_t add -- so every P_t is bit-identical to the reference fp32
sequence.

Per-step engine assignment (v3):
    VectorE op1: custom fused DVE op LIF_STEP_ANT
                 out = (Src0 - C0*(Src0 > C1)) + Src1   (reset + add, 1x)
    VectorE op2: tensor_scalar is_gt -> u8 spike        (single-src, 2x)
    HWDGE: x loads; u8 spike stores (4x less write traffic; host upcasts).
The serial recurrence is pipelined across G independent column groups.
"""

import os

import numpy as np

T, B, C, H, W = 32, 16, 128, 32, 32
NCORES = 8
BPC = B // NCORES          # batches per core
P = 128                    # SBUF partitions
F = (BPC * C * H * W) // P # 2048 free-dim elements per step
VER = os.environ.get("LIF_VERSION", "v3")
G = int(os.environ.get("LIF_GROUPS", "1"))   # column groups (v3: serial chain
FG = F // G                                  # already short; 1 is fine)

_cache = {}


def _prevfloat(v: float) -> float:
    return float(np.nextafter(np.float32(v), np.float32(0)))


def _register_lif_op():
    """Append the fused LIF step op to the custom-DVE registry (idempotent).

    out = (Src0 - s0 * (Src0 > s1)) + Src1
    s0 = 4^(t-1) reset amount, s1 = prevfloat(4^(t-1)) so the strict > equals
    the reference's >= on fp32 values.
    """
    from concourse import dve_ops
    from concourse.dve_spec import Spec, Src0, Src1, C0, C1, lower, _has_src1
    from concourse.dve_uop import DveOpSpec

    for op in dve_ops.OPS:
        if op.name == "LIF_STEP_ANT":
            return op

    spec = Spec(
        body=(Src0 - C0 * (Src0 > C1)) + Src1,
        reference=lambda in0, in1, s0, s1, imm2: (
            in0 - s0 * (in0 > s1).astype(np.float32)
        )
        + in1,
    )
    op = dve_ops.DveOp("LIF_STEP_ANT", spec, subdim=False, uops_sha={})
    dve_ops.OPS.append(op)
    dve_ops.CUSTOM_DVE_SPECS[op.name] = op.spec
    dve_ops._SUB_OPCODE_FOR_NAME[op.name] = (
        dve_ops._CUSTOM_DVE_ROW_BASE + len(dve_ops.OPS) - 1
    )
    # self-pin the sha exactly the way DveOp.compile() derives it
    for ver in ("v3", "v4"):
        try:
            compiled = DveOpSpec(
                name=op.name,
                opcode=dve_ops.get_dve_sub_opcode(op.name),
                uops=lower(spec, ver=ver),
                rd1_en=_has_src1(spec),
            )
            op.uops_sha[ver] = compiled.sha(ver)
        except Exception:
            pass
    return op


def _build(reps: int = 1):
    import concourse.bacc as bacc
    import concourse.mybir as mybir
    from concourse.tile import TileContext

    nc = bacc.Bacc(None, target_bir_lowering=False)
    x_d = nc.dram_tensor("x", [T, P, F], mybir.dt.float32, kind="ExternalInput")
    o_d = nc.dram_tensor("o", [T, P, F], mybir.dt.uint8, kind="ExternalOutput")

    fp32 = mybir.dt.float32
    u8 = mybir.dt.uint8
    Alu = mybir.AluOpType
    Act = mybir.ActivationFunctionType
    lif = _register_lif_op() if VER == "v3" else None

    with TileContext(nc) as tc:
        with (
            tc.tile_pool(name="mem", bufs=1) as mempool,
            tc.tile_pool(name="xin", bufs=6) as xpool,
            tc.tile_pool(name="spk", bufs=3) as spool,
            tc.tile_pool(name="out", bufs=3) as opool,
        ):
            if VER == "v3":
                p_st = [
                    mempool.tile([P, FG], fp32, name=f"p{g}", tag=f"p{g}")
                    for g in range(G)
                ]
                for _ in range(reps):  # reps>1 only for benchmarking
                    for g in range(G):
                        nc.vector.memset(p_st[g], 0.0)
                    for t in range(T):
                        # reset uses the PREVIOUS step's threshold; t=0 resets
                        # nothing (state is 0), s0=0 makes the op a plain add
                        rst = float(4.0 ** (t - 1)) if t > 0 else 0.0
                        rthr = _prevfloat(4.0 ** (t - 1)) if t > 0 else 1.0
                        sthr = _prevfloat(4.0 ** t)
                        for g in range(G):
                            cols = slice(g * FG, (g + 1) * FG)
                            xt = xpool.tile([P, FG], fp32, name=f"x{g}", tag=f"x{g}")
                            nc.sync.dma_start(out=xt, in_=x_d[t][:, cols])
                            # P = (P - 4^{t-1}*(P > prev4^{t-1})) + y_t
                            nc.vector._custom_dve(
                                lif, out=p_st[g], in0=p_st[g], in1=xt,
                                s0=rst, s1=rthr,
                            )
                            # spike: u8 = (P > prevfloat(4^t))  ==  (P >= 4^t)
                            su = opool.tile([P, FG], u8, name=f"u{g}", tag=f"u{g}")
                            nc.vector.tensor_scalar(
                                su, p_st[g], sthr, None, Alu.is_gt
                            )
                            nc.sync.dma_start(out=o_d[t][:, cols], in_=su)
            else:  # v1 fallback: plain-threshold STT/TS/TT (no host pre-scale)
                m = mempool.tile([P, F], fp32, name="m", tag="m")
                for _ in range(reps):
                    nc.vector.memset(m, 0.0)
                    for t in range(T):
                        xt = xpool.tile([P, F], fp32, name="x", tag="x")
                        nc.sync.dma_start(out=xt, in_=x_d[t])
                        nc.vector.scalar_tensor_tensor(
                            m, m, 0.25, xt, Alu.mult, Alu.add
                        )
                        s = spool.tile([P, F], fp32, name="s", tag="s")
                        nc.vector.tensor_scalar(s, m, 1.0, None, Alu.is_ge)
                        su = opool.tile([P, F], u8, name="su", tag="su")
                        nc.scalar.activation(su, s, Act.Copy, bias=0.0, scale=1.0)
                        nc.sync.dma_start(out=o_d[t], in_=su)
                        nc.vector.tensor_tensor(m, m, s, Alu.subtract)
    nc.finalize()
    return nc


def kernel(x: np.ndarray) -> np.ndarray:
    from concourse.bass_utils import run_bass_kernel_spmd

    assert x.shape == (T, B, C, H, W) and x.dtype == np.float32
    if "nc" not in _cache:
        _cache["nc"] = _build()
    nc = _cache["nc"]

    if VER == "v3":
        # host-side pre-scale: y_t = 4^t * x_t (exact power-of-two multiply)
        scale = (4.0 ** np.arange(T, dtype=np.float64)).astype(np.float32)
        y = x * scale[:, None, None, None, None]
    else:
        y = x

    in_maps = []
    for k in range(NCORES):
        yk = np.ascontiguousarray(y[:, k * BPC : (k + 1) * BPC]).reshape(T, P, F)
        in_maps.append({"x": yk})

    res = run_bass_kernel_spmd(nc, in_maps, core_ids=list(range(NCORES)))
    _cache["last_result"] = res

    out = np.empty((T, B, C, H, W), dtype=np.float32)
    for k in range(NCORES):
        ok = res.results[k]["o"].reshape(T, BPC, C, H, W)
        out[:, k * BPC : (k + 1) * BPC] = ok.astype(np.float32)
    return out
